# revision 42
# baseline (speedup 1.0000x reference)
"""Trainium2 Bass kernel for nn_NetworkStitch_5016521802529.

Cost-volume block: f1n = l2_normalize_c(feature1); hori/verti 9-offset
correlation bands vs feature2; leaky_relu; concat -> [B, 18, H, W].

Strategy (pure data-parallel over batch, 32 samples/core on 8 cores):
  - SWDGE casting DMA loads f32 DRAM -> bf16 SBUF, 4 samples per
    [128, 2, H, W] tile (sample half on partition halves, pair slot on
    dim 1) -- 4 MB reads amortize the ~2us SWDGE fixed cost.
  - s^2 = sum_c f1^2 via DVE square + ones-stationary matmul (output
    replicated across partitions); single ACT Abs_reciprocal_sqrt
    (scale=64^2) gives s_inv = 1/(64*s) in the same layout.
  - f1n = f1b * s_inv (bf16).
  - Per h (hori) / per w (verti): single-line Gram G = f1n_l^T @ f2b_l
    ([64,64], K=64) on 64x64 PE quadrants; the two samples' matmuls are
    interleaved so consecutive LDWEIGHTS hit different row groups (PE
    pulls them ahead of in-flight MATMULs). 16 Grams per PSUM bank
    [128, 512] (partition half = line%2, free slot = (line%16)//2,
    bank = line//16).
  - Eviction PSUM->SBUF fused with leaky-relu: banks 0-1 via ACT Lrelu
    (alpha=.01), banks 2-3 via DVE copy + scalar_tensor_tensor --
    balances the two engines.
  - Per sample one [128, 2, 2048] bf16 strip (both directions) stored
    with a single 1 MB DMA; host extracts the 9-diagonal bands (pure
    indexing) during unshard.
"""

import os
import sys

sys.path.insert(0, "/opt/trn_rl_repo")

import numpy as np
import ml_dtypes

import concourse.bacc as bacc
import concourse.bass as bass
import concourse.tile as tile
from concourse import mybir

B, C, H, W = 256, 64, 64, 64
N_CORES = 8
B_CORE = B // N_CORES
SR = 4
ND = 2 * SR + 1  # 9
HW = H * W
BF16 = mybir.dt.bfloat16
F32 = mybir.dt.float32

# Eviction split: banks < this evicted by ACT(Lrelu), rest by DVE
# (copy + stt; stt cannot read two PSUM operands).
ACT_EVICT_BANKS = 2


def build_nc(bcore=B_CORE, act_evict_banks=ACT_EVICT_BANKS):
    """Build the per-core Bass graph for `bcore` samples (mult of 4)."""
    assert bcore % 4 == 0
    nc = bacc.Bacc("TRN2", target_bir_lowering=False, debug=False)
    # Inputs staged as bf16 by the host (bit-identical to the bf16 cast
    # the kernel would do on load anyway) -- halves input HBM traffic
    # and lets the loads use HWDGE instead of casting SWDGE.
    f1d = nc.dram_tensor("f1", [bcore, C, H, W], BF16, kind="ExternalInput")
    f2d = nc.dram_tensor("f2", [bcore, C, H, W], BF16, kind="ExternalInput")
    # per pair: [128 partitions, 2 samples, 2 directions, 2048] bf16 strips
    outd = nc.dram_tensor(
        "out", [bcore // 2, 128, 2, 2, 4 * 512], BF16, kind="ExternalOutput"
    )
    ngrp = bcore // 4

    from contextlib import ExitStack

    with tile.TileContext(nc) as tc, ExitStack() as ctx:
        io = ctx.enter_context(tc.tile_pool(name="io", bufs=2))
        work = ctx.enter_context(tc.tile_pool(name="work", bufs=2))
        strips = ctx.enter_context(tc.tile_pool(name="strips", bufs=2))
        const = ctx.enter_context(tc.tile_pool(name="const", bufs=1))
        gram = ctx.enter_context(tc.tile_pool(name="gram", bufs=3, space="PSUM"))
        nrm = ctx.enter_context(tc.tile_pool(name="nrm", bufs=1, space="PSUM"))

        ones_t = const.tile([128, 64], BF16)
        nc.vector.memset(ones_t[:], 1.0)
        alpha01 = const.tile([128, 1], F32)
        nc.vector.memset(alpha01[:], 0.01)

        for grp in range(ngrp):
            # ---- casting loads: 4 samples -> [128, 2, H, W] bf16 ----
            f1b = io.tile([128, 2, H, W], BF16, tag="f1b")
            f2b = io.tile([128, 2, H, W], BF16, tag="f2b")
            if grp == 0:
                # split the cold-start f1 load per pair so pair-0 compute
                # can begin while the rest of the group streams in
                for j in range(2):
                    nc.sync.dma_start(
                        out=f1b[:, j],
                        in_=f1d[2 * j : 2 * j + 2].rearrange(
                            "half c h w -> (half c) h w", half=2
                        ),
                    )
            else:
                nc.sync.dma_start(
                    out=f1b[:],
                    in_=f1d[4 * grp : 4 * grp + 4].rearrange(
                        "(j half) c h w -> (half c) j h w", j=2, half=2
                    ),
                )
            nc.sync.dma_start(
                out=f2b[:],
                in_=f2d[4 * grp : 4 * grp + 4].rearrange(
                    "(j half) c h w -> (half c) j h w", j=2, half=2
                ),
            )

            # ---- norms for both pairs first (batches the ACT rsqrt ops
            # so the activation table only switches twice per group) ----
            f1ns = []
            for j in range(2):  # pair slot within the 4-sample group
                f1sq = work.tile([128, HW], BF16, name=f"f1sq{j}",
                                 tag=f"f1sq{j}")
                nc.vector.tensor_mul(
                    f1sq[:],
                    f1b[:, j].rearrange("p h w -> p (h w)"),
                    f1b[:, j].rearrange("p h w -> p (h w)"),
                )
                s_inv = work.tile([128, HW], BF16, name=f"sinv{j}",
                                  tag=f"sinv{j}")
                for ch2 in range(4):  # 2 chunks per 2-bank PSUM tile
                    ps = nrm.tile([128, 1024], F32, tag="nps")
                    for sub in range(2):
                        sl = slice((2 * ch2 + sub) * 512,
                                   (2 * ch2 + sub + 1) * 512)
                        psl = slice(sub * 512, (sub + 1) * 512)
                        for half in range(2):
                            po = half * 64
                            nc.tensor.matmul(
                                out=ps[po : po + 64, psl],
                                lhsT=ones_t[po : po + 64, :],
                                rhs=f1sq[po : po + 64, sl],
                                tile_position=(po, po),
                            )
                    # 1/sqrt(64^2 * ps) = 1/(64*s) in one ACT op (sum >=
                    # 0 so the |x| in Abs_reciprocal_sqrt is a no-op).
                    nc.scalar.activation(
                        out=s_inv[:, ch2 * 1024 : (ch2 + 1) * 1024],
                        in_=ps[:],
                        func=mybir.ActivationFunctionType.Abs_reciprocal_sqrt,
                        scale=float(64 * 64),
                    )

                f1n = work.tile([128, H, W], BF16, name=f"f1n{j}",
                                tag=f"f1n{j}")
                nc.vector.tensor_mul(
                    f1n[:].rearrange("p h w -> p (h w)"),
                    f1b[:, j].rearrange("p h w -> p (h w)"),
                    s_inv[:],
                )
                f1ns.append(f1n)

            for j in range(2):
                f1n = f1ns[j]
                # ---- Grams (samples interleaved for LDW overlap) ----
                st = strips.tile([128, 2, 2, 4 * 512], BF16, tag="strip")
                for direction in range(2):  # 0 = hori (per h), 1 = verti
                    for g in range(4):  # bank group of 16 lines
                        ps2 = [
                            gram.tile([128, 512], F32,
                                      name=f"gps{s}", tag=f"gps{s}")
                            for s in range(2)
                        ]
                        for i in range(16):
                            line = g * 16 + i
                            half, slot = i % 2, i // 2
                            for smp in range(2):
                                ko = smp * 64
                                if direction == 0:
                                    lhsT = f1n[ko : ko + 64, line, :]
                                    rhs = f2b[ko : ko + 64, j, line, :]
                                else:
                                    lhsT = f1n[ko : ko + 64, :, line]
                                    rhs = f2b[ko : ko + 64, j, :, line]
                                nc.tensor.matmul(
                                    out=ps2[smp][
                                        half * 64 : half * 64 + 64,
                                        slot * 64 : slot * 64 + 64,
                                    ],
                                    lhsT=lhsT,
                                    rhs=rhs,
                                    tile_position=(ko, half * 64),
                                )
                        osl = slice(g * 512, (g + 1) * 512)
                        for smp in range(2):
                            dst = st[:, smp, direction, osl]
                            if g < act_evict_banks or (
                                g == 3 and direction == 0
                            ):
                                # ACT fused evict + leaky from PSUM
                                nc.scalar.activation(
                                    out=dst, in_=ps2[smp][:],
                                    func=mybir.ActivationFunctionType.Lrelu,
                                    alpha=alpha01[:],
                                )
                            else:
                                # DVE evict: copy PSUM->SBUF bf16; leaky
                                # applied afterwards over the whole DVE
                                # span in one stt op per (smp, dir).
                                nc.vector.tensor_copy(
                                    out=dst, in_=ps2[smp][:]
                                )
                    # one batched leaky pass over this direction's
                    # DVE-evicted span (stt cannot take two PSUM reads)
                    dlo = act_evict_banks * 512
                    dhi = (4 if direction == 1 else 3) * 512
                    for smp in range(2):
                        dvs = st[:, smp, direction, dlo:dhi]
                        nc.vector.scalar_tensor_tensor(
                            out=dvs,
                            in0=dvs,
                            scalar=0.01,
                            in1=dvs,
                            op0=mybir.AluOpType.mult,
                            op1=mybir.AluOpType.max,
                        )
                nc.sync.dma_start(out=outd[2 * grp + j], in_=st[:])

    nc.compile()
    return nc


_NC_CACHE = {}


def _get_nc(bcore=B_CORE):
    if bcore not in _NC_CACHE:
        _NC_CACHE[bcore] = build_nc(bcore)
    return _NC_CACHE[bcore]


def _extract_bands(strips):
    """strips: [bcore//2, 128, 2, 2, 2048] float32-ish -> [bcore, 18, H, W].

    Gram line L (h for hori, w for verti) of sample 2*pr+smp: G_L[r, c] =
      strips[pr, (L%2)*64 + r, smp, dir, (L//16)*512 + ((L%16)//2)*64 + c].
    hori[d, h, w] = G_h[w, w+d-4]; verti[d, h, w] = Gv_w[h, h+d-4].
    """
    bcore = strips.shape[0] * 2
    s = np.asarray(strips, dtype=np.float32)
    # [pr, half(2), r(64), smp(2), dir(2), bank(4), slot(8), c(64)]
    s = s.reshape(bcore // 2, 2, 64, 2, 2, 4, 8, 64)
    # line index L = bank*16 + slot*2 + half -> G[(pr, smp), dir, L, r, c]
    g = s.transpose(0, 3, 4, 5, 6, 1, 2, 7).reshape(bcore, 2, 64, 64, 64)
    out = np.zeros((bcore, 2, ND, 64, 64), dtype=np.float32)
    idx = np.arange(64)
    for d in range(ND):
        o = d - SR
        lo, hi = max(0, -o), min(64, 64 - o)
        r = idx[lo:hi]
        # advanced idxs (incl. the int) are slice-separated -> dims lead:
        # result [len(r), b, L]
        hvals = g[:, 0, :, r, r + o]  # [w-valid, b, h=L]
        vvals = g[:, 1, :, r, r + o]  # [h-valid, b, w=L]
        out[:, 0, d, :, lo:hi] = hvals.transpose(1, 2, 0)
        out[:, 1, d, lo:hi, :] = vvals.transpose(1, 0, 2)
    return out.reshape(bcore, 2 * ND, 64, 64)


def kernel(feature1, feature2, search_range):
    assert int(search_range) == SR
    f1 = np.ascontiguousarray(
        np.asarray(feature1, dtype=np.float32).astype(ml_dtypes.bfloat16)
    )
    f2 = np.ascontiguousarray(
        np.asarray(feature2, dtype=np.float32).astype(ml_dtypes.bfloat16)
    )
    bcore = f1.shape[0] // N_CORES
    nc = _get_nc(bcore)

    from concourse.bass_utils import run_bass_kernel_spmd

    in_maps = [
        {
            "f1": f1[c * bcore : (c + 1) * bcore],
            "f2": f2[c * bcore : (c + 1) * bcore],
        }
        for c in range(N_CORES)
    ]
    res = run_bass_kernel_spmd(nc, in_maps, list(range(N_CORES)))
    outs = [
        _extract_bands(res.results[c]["out"].astype(np.float32))
        for c in range(N_CORES)
    ]
    return np.concatenate(outs, axis=0)


# revision 44
# speedup vs baseline: 1.1536x; 1.1536x over previous
"""Trainium2 Bass kernel for nn_NetworkStitch_5016521802529.

Cost-volume block: f1n = l2_normalize_c(feature1); hori/verti 9-offset
correlation bands vs feature2; leaky_relu; concat -> [B, 18, H, W].

Strategy (pure data-parallel over batch, 32 samples/core on 8 cores):
  - SWDGE casting DMA loads f32 DRAM -> bf16 SBUF, 4 samples per
    [128, 2, H, W] tile (sample half on partition halves, pair slot on
    dim 1) -- 4 MB reads amortize the ~2us SWDGE fixed cost.
  - s^2 = sum_c f1^2 via DVE square + ones-stationary matmul (output
    replicated across partitions); single ACT Abs_reciprocal_sqrt
    (scale=64^2) gives s_inv = 1/(64*s) in the same layout.
  - f1n = f1b * s_inv (bf16).
  - Per h (hori) / per w (verti): single-line Gram G = f1n_l^T @ f2b_l
    ([64,64], K=64) on 64x64 PE quadrants; the two samples' matmuls are
    interleaved so consecutive LDWEIGHTS hit different row groups (PE
    pulls them ahead of in-flight MATMULs). 16 Grams per PSUM bank
    [128, 512] (partition half = line%2, free slot = (line%16)//2,
    bank = line//16).
  - Eviction PSUM->SBUF fused with leaky-relu: banks 0-1 via ACT Lrelu
    (alpha=.01), banks 2-3 via DVE copy + scalar_tensor_tensor --
    balances the two engines.
  - Per sample one [128, 2, 2048] bf16 strip (both directions) stored
    with a single 1 MB DMA; host extracts the 9-diagonal bands (pure
    indexing) during unshard.
"""

import os
import sys

sys.path.insert(0, "/opt/trn_rl_repo")

import numpy as np
import ml_dtypes

import concourse.bacc as bacc
import concourse.bass as bass
import concourse.tile as tile
from concourse import mybir

B, C, H, W = 256, 64, 64, 64
N_CORES = 8
B_CORE = B // N_CORES
SR = 4
ND = 2 * SR + 1  # 9
HW = H * W
BF16 = mybir.dt.bfloat16
F32 = mybir.dt.float32

# Eviction split: banks < this evicted by ACT(Lrelu), rest by DVE
# (copy + stt; stt cannot read two PSUM operands).
ACT_EVICT_BANKS = 2


def build_nc(bcore=B_CORE, act_evict_banks=ACT_EVICT_BANKS):
    """Build the per-core Bass graph for `bcore` samples (mult of 4)."""
    assert bcore % 4 == 0
    nc = bacc.Bacc("TRN2", target_bir_lowering=False, debug=False)
    # Inputs staged as bf16 by the host (bit-identical to the bf16 cast
    # the kernel would do on load anyway) -- halves input HBM traffic
    # and lets the loads use HWDGE instead of casting SWDGE.
    f1d = nc.dram_tensor("f1", [bcore, C, H, W], BF16, kind="ExternalInput")
    f2d = nc.dram_tensor("f2", [bcore, C, H, W], BF16, kind="ExternalInput")
    # per pair: [128 partitions, 2 samples, 2 directions, 2048] bf16 strips
    outd = nc.dram_tensor(
        "out", [bcore // 2, 128, 2, 2, 4 * 512], BF16, kind="ExternalOutput"
    )
    ngrp = bcore // 4

    from contextlib import ExitStack

    with tile.TileContext(nc) as tc, ExitStack() as ctx:
        io = ctx.enter_context(tc.tile_pool(name="io", bufs=2))
        work = ctx.enter_context(tc.tile_pool(name="work", bufs=2))
        strips = ctx.enter_context(tc.tile_pool(name="strips", bufs=2))
        const = ctx.enter_context(tc.tile_pool(name="const", bufs=1))
        gram = ctx.enter_context(tc.tile_pool(name="gram", bufs=3, space="PSUM"))
        nrm = ctx.enter_context(tc.tile_pool(name="nrm", bufs=2, space="PSUM"))

        ones_t = const.tile([128, 64], BF16)
        nc.vector.memset(ones_t[:], 1.0)
        alpha01 = const.tile([128, 1], F32)
        nc.vector.memset(alpha01[:], 0.01)

        for grp in range(ngrp):
            # ---- casting loads: 4 samples -> [128, 2, H, W] bf16 ----
            f1b = io.tile([128, 2, H, W], BF16, tag="f1b")
            f2b = io.tile([128, 2, H, W], BF16, tag="f2b")
            if grp == 0:
                # split the cold-start f1 load per pair so pair-0 compute
                # can begin while the rest of the group streams in
                for j in range(2):
                    nc.sync.dma_start(
                        out=f1b[:, j],
                        in_=f1d[2 * j : 2 * j + 2].rearrange(
                            "half c h w -> (half c) h w", half=2
                        ),
                    )
            else:
                nc.sync.dma_start(
                    out=f1b[:],
                    in_=f1d[4 * grp : 4 * grp + 4].rearrange(
                        "(j half) c h w -> (half c) j h w", j=2, half=2
                    ),
                )
            nc.sync.dma_start(
                out=f2b[:],
                in_=f2d[4 * grp : 4 * grp + 4].rearrange(
                    "(j half) c h w -> (half c) j h w", j=2, half=2
                ),
            )

            # ---- norms for both pairs first (batches the ACT rsqrt ops
            # so the activation table only switches twice per group) ----
            f1ns = []
            for j in range(2):  # pair slot within the 4-sample group
                f1sq = work.tile([128, HW], BF16, name=f"f1sq{j}",
                                 tag=f"f1sq{j}")
                nc.vector.tensor_mul(
                    f1sq[:],
                    f1b[:, j].rearrange("p h w -> p (h w)"),
                    f1b[:, j].rearrange("p h w -> p (h w)"),
                )
                s_inv = work.tile([128, HW], BF16, name=f"sinv{j}",
                                  tag=f"sinv{j}")
                for ch in range(8):
                    ps = nrm.tile([128, 512], F32, tag="nps")
                    sl = slice(ch * 512, (ch + 1) * 512)
                    for half in range(2):
                        po = half * 64
                        nc.tensor.matmul(
                            out=ps[po : po + 64, :],
                            lhsT=ones_t[po : po + 64, :],
                            rhs=f1sq[po : po + 64, sl],
                            tile_position=(po, po),
                        )
                    # 1/sqrt(64^2 * ps) = 1/(64*s) in one ACT op (sum >=
                    # 0 so the |x| in Abs_reciprocal_sqrt is a no-op).
                    nc.scalar.activation(
                        out=s_inv[:, sl], in_=ps[:],
                        func=mybir.ActivationFunctionType.Abs_reciprocal_sqrt,
                        scale=float(64 * 64),
                    )

                f1n = work.tile([128, H, W], BF16, name=f"f1n{j}",
                                tag=f"f1n{j}")
                nc.vector.tensor_mul(
                    f1n[:].rearrange("p h w -> p (h w)"),
                    f1b[:, j].rearrange("p h w -> p (h w)"),
                    s_inv[:],
                )
                f1ns.append(f1n)

            for j in range(2):
                f1n = f1ns[j]
                # ---- Grams (samples interleaved for LDW overlap) ----
                st = strips.tile([128, 2, 2, 4 * 512], BF16, tag="strip")
                for direction in range(2):  # 0 = hori (per h), 1 = verti
                    for g in range(4):  # bank group of 16 lines
                        ps2 = [
                            gram.tile([128, 512], F32,
                                      name=f"gps{s}", tag=f"gps{s}")
                            for s in range(2)
                        ]
                        for i in range(16):
                            line = g * 16 + i
                            half, slot = i % 2, i // 2
                            for smp in range(2):
                                ko = smp * 64
                                if direction == 0:
                                    lhsT = f1n[ko : ko + 64, line, :]
                                    rhs = f2b[ko : ko + 64, j, line, :]
                                else:
                                    lhsT = f1n[ko : ko + 64, :, line]
                                    rhs = f2b[ko : ko + 64, j, :, line]
                                nc.tensor.matmul(
                                    out=ps2[smp][
                                        half * 64 : half * 64 + 64,
                                        slot * 64 : slot * 64 + 64,
                                    ],
                                    lhsT=lhsT,
                                    rhs=rhs,
                                    tile_position=(ko, half * 64),
                                )
                        osl = slice(g * 512, (g + 1) * 512)
                        for smp in range(2):
                            dst = st[:, smp, direction, osl]
                            if g < act_evict_banks or (
                                g == 3 and direction == 0
                            ):
                                # ACT fused evict + leaky from PSUM
                                nc.scalar.activation(
                                    out=dst, in_=ps2[smp][:],
                                    func=mybir.ActivationFunctionType.Lrelu,
                                    alpha=alpha01[:],
                                )
                            else:
                                # DVE evict: copy PSUM->SBUF bf16; leaky
                                # applied afterwards over the whole DVE
                                # span in one stt op per (smp, dir).
                                nc.vector.tensor_copy(
                                    out=dst, in_=ps2[smp][:]
                                )
                    # one batched leaky pass over this direction's
                    # DVE-evicted span (stt cannot take two PSUM reads)
                    dlo = act_evict_banks * 512
                    dhi = (4 if direction == 1 else 3) * 512
                    for smp in range(2):
                        dvs = st[:, smp, direction, dlo:dhi]
                        nc.vector.scalar_tensor_tensor(
                            out=dvs,
                            in0=dvs,
                            scalar=0.01,
                            in1=dvs,
                            op0=mybir.AluOpType.mult,
                            op1=mybir.AluOpType.max,
                        )
                nc.sync.dma_start(out=outd[2 * grp + j], in_=st[:])

    nc.compile()
    return nc


_NC_CACHE = {}


def _get_nc(bcore=B_CORE):
    if bcore not in _NC_CACHE:
        _NC_CACHE[bcore] = build_nc(bcore)
    return _NC_CACHE[bcore]


def _extract_bands(strips):
    """strips: [bcore//2, 128, 2, 2, 2048] float32-ish -> [bcore, 18, H, W].

    Gram line L (h for hori, w for verti) of sample 2*pr+smp: G_L[r, c] =
      strips[pr, (L%2)*64 + r, smp, dir, (L//16)*512 + ((L%16)//2)*64 + c].
    hori[d, h, w] = G_h[w, w+d-4]; verti[d, h, w] = Gv_w[h, h+d-4].
    """
    bcore = strips.shape[0] * 2
    s = np.asarray(strips, dtype=np.float32)
    # [pr, half(2), r(64), smp(2), dir(2), bank(4), slot(8), c(64)]
    s = s.reshape(bcore // 2, 2, 64, 2, 2, 4, 8, 64)
    # line index L = bank*16 + slot*2 + half -> G[(pr, smp), dir, L, r, c]
    g = s.transpose(0, 3, 4, 5, 6, 1, 2, 7).reshape(bcore, 2, 64, 64, 64)
    out = np.zeros((bcore, 2, ND, 64, 64), dtype=np.float32)
    idx = np.arange(64)
    for d in range(ND):
        o = d - SR
        lo, hi = max(0, -o), min(64, 64 - o)
        r = idx[lo:hi]
        # advanced idxs (incl. the int) are slice-separated -> dims lead:
        # result [len(r), b, L]
        hvals = g[:, 0, :, r, r + o]  # [w-valid, b, h=L]
        vvals = g[:, 1, :, r, r + o]  # [h-valid, b, w=L]
        out[:, 0, d, :, lo:hi] = hvals.transpose(1, 2, 0)
        out[:, 1, d, lo:hi, :] = vvals.transpose(1, 0, 2)
    return out.reshape(bcore, 2 * ND, 64, 64)


def kernel(feature1, feature2, search_range):
    assert int(search_range) == SR
    f1 = np.ascontiguousarray(
        np.asarray(feature1, dtype=np.float32).astype(ml_dtypes.bfloat16)
    )
    f2 = np.ascontiguousarray(
        np.asarray(feature2, dtype=np.float32).astype(ml_dtypes.bfloat16)
    )
    bcore = f1.shape[0] // N_CORES
    nc = _get_nc(bcore)

    from concourse.bass_utils import run_bass_kernel_spmd

    in_maps = [
        {
            "f1": f1[c * bcore : (c + 1) * bcore],
            "f2": f2[c * bcore : (c + 1) * bcore],
        }
        for c in range(N_CORES)
    ]
    res = run_bass_kernel_spmd(nc, in_maps, list(range(N_CORES)))
    outs = [
        _extract_bands(res.results[c]["out"].astype(np.float32))
        for c in range(N_CORES)
    ]
    return np.concatenate(outs, axis=0)
